# revision 1
# baseline (speedup 1.0000x reference)
"""Grouped-expert SwiGLU FFN (MoE) kernel for 8 Trainium2 NeuronCores.

Problem: 8 experts, tokens pre-sorted into contiguous equal segments.
  sorted_x: (8192, 512) f32, w12: (8, 2048, 512) f32, w3: (8, 512, 1024) f32
  out[t] = SwiGLU(x[t] @ w12[e].T) @ w3[e].T   for t in expert e's segment

Sharding: expert parallelism. Core e gets its 1024-token segment plus
w12[e]/w3[e]; no collectives. Host pre-transposes operands so the
contraction dim lands on SBUF partitions (no on-chip transposes):

  GEMM1 (contract d_model=512, 4 chunks of 128):
    lhsT = w12[e].T chunk (i=128, j=128)  [stationary]
    rhs  = x_seg.T  chunk (i=128, t=512)  [moving]
    psum (j=128, t=512) accumulated over 4 i-chunks  -> h12^T orientation
  SwiGLU: silu(h1^T) * h2^T elementwise in (j, t) layout (ACT + DVE)
  GEMM2 (contract hidden=1024, 8 chunks of 128):
    lhsT = h^T chunk   (j=128, t=128)  [stationary]
    rhs  = w3[e].T chunk (j=128, o=512) [moving]
    psum (t=128, o=512) accumulated over 8 j-chunks -> natural (t, o) output

Matmuls run as float32r (fp32 data, reduced-precision multiply at full
PE rate; plain fp32 matmul is 4x slower on TRN2). Inputs are packed on
the host so w12 loads as one DMA per chunk-pair and w3 as two group
DMAs (the HWDGE per-instruction fixed cost ~0.6us serializes on the
ring); input tiles are double-buffered so back-to-back invocations
overlap; output stores go out on the Activation HWDGE ring to keep the
SP ring free for input loads.
"""

import numpy as np

N_EXPERTS = 8
D_MODEL = 512
HIDDEN = 1024
TOKENS_PER_EXPERT = 1024
N_CORES = 8

_CACHE = {}


def _build_program(repeat=1, dtype="f32r"):
    import concourse.mybir as mybir
    import concourse.tile as tile
    from concourse import bacc

    f32 = mybir.dt.float32
    din = {"f32r": mybir.dt.float32r, "bf16": mybir.dt.bfloat16,
           "fp16": mybir.dt.float16}[dtype]
    P = 128
    IC = D_MODEL // P            # 4 chunks of d_model
    JC = HIDDEN // P             # 8 chunk-pairs of 2*hidden / chunks of hidden
    WG = 2                       # w3 DMA groups
    TB = 512                     # token block (moving free dim, fp32 max)
    NT = TOKENS_PER_EXPERT // TB  # 2 token blocks
    TM = TOKENS_PER_EXPERT // P  # 8 output token chunks

    nc = bacc.Bacc(None, target_bir_lowering=False)

    # host-packed layouts (see _pack_inputs)
    xT_d = nc.dram_tensor("xT", [IC, P, TOKENS_PER_EXPERT], din, kind="ExternalInput")
    w12_d = nc.dram_tensor("w12p", [JC, 2, P, IC * P], din, kind="ExternalInput")
    w3_d = nc.dram_tensor("w3g", [WG, JC // WG, P, D_MODEL], din, kind="ExternalInput")
    out_d = nc.dram_tensor("out", [TOKENS_PER_EXPERT, D_MODEL], f32, kind="ExternalOutput")

    with tile.TileContext(nc) as tc:
        with (
            tc.tile_pool(name="persist", bufs=2) as persist,
            tc.tile_pool(name="work", bufs=3) as work,
            tc.tile_pool(name="ps1", bufs=3, space="PSUM") as ps1,
            tc.tile_pool(name="ps2", bufs=2, space="PSUM") as ps2,
        ):
            for it in range(repeat):
                # ---- loads: issue order = consumption order ----
                xt, w12, w3 = [], [], []
                for ic in range(IC):
                    t = persist.tile([P, TOKENS_PER_EXPERT], din, tag=f"xt{ic}",
                                     name=f"i{it}_xt{ic}")
                    nc.sync.dma_start(t[:], xT_d[ic])
                    xt.append(t)
                    if ic == 0:
                        t = persist.tile([P, 2, IC * P], din, tag="w12_0",
                                         name=f"i{it}_w12_0")
                        nc.sync.dma_start(t[:], w12_d[0].rearrange("h p m -> p h m"))
                        w12.append(t)
                for p in range(1, JC):
                    t = persist.tile([P, 2, IC * P], din, tag=f"w12_{p}",
                                     name=f"i{it}_w12_{p}")
                    nc.sync.dma_start(t[:], w12_d[p].rearrange("h p m -> p h m"))
                    w12.append(t)
                for g in range(WG):
                    t = persist.tile([P, JC // WG, D_MODEL], din, tag=f"w3_{g}",
                                     name=f"i{it}_w3_{g}")
                    nc.sync.dma_start(t[:], w3_d[g].rearrange("q p m -> p q m"))
                    w3.append(t)
                h = [persist.tile([P, TOKENS_PER_EXPERT], din, tag=f"h{p}", bufs=1,
                                  name=f"i{it}_h{p}") for p in range(JC)]

                # ---- GEMM1 + SwiGLU ----
                for p in range(JC):
                    for tb in range(NT):
                        tsl = slice(tb * TB, (tb + 1) * TB)
                        ps_a = ps1.tile([P, TB], f32, tag="ps_a", name=f"i{it}_a{p}_{tb}")
                        ps_b = ps1.tile([P, TB], f32, tag="ps_b", name=f"i{it}_b{p}_{tb}")
                        for ic in range(IC):
                            nc.tensor.matmul(
                                ps_a[:], w12[p][:, 0, ic * P:(ic + 1) * P],
                                xt[ic][:, tsl], start=(ic == 0), stop=(ic == IC - 1))
                        for ic in range(IC):
                            nc.tensor.matmul(
                                ps_b[:], w12[p][:, 1, ic * P:(ic + 1) * P],
                                xt[ic][:, tsl], start=(ic == 0), stop=(ic == IC - 1))
                        s = work.tile([P, TB], f32, tag="silu", name=f"i{it}_s{p}_{tb}")
                        nc.scalar.activation(s[:], ps_a[:],
                                             mybir.ActivationFunctionType.Silu)
                        nc.vector.tensor_tensor(h[p][:, tsl], s[:], ps_b[:],
                                                mybir.AluOpType.mult)

                # ---- GEMM2 ----
                for tm in range(TM):
                    ps_o = ps2.tile([P, D_MODEL], f32, tag="ps_o", name=f"i{it}_o{tm}")
                    for jc in range(JC):
                        nc.tensor.matmul(
                            ps_o[:], h[jc][:, tm * P:(tm + 1) * P],
                            w3[jc // (JC // WG)][:, jc % (JC // WG), :],
                            start=(jc == 0), stop=(jc == JC - 1))
                    o = work.tile([P, D_MODEL], f32, tag="o", name=f"i{it}_oo{tm}")
                    nc.vector.tensor_copy(o[:], ps_o[:])
                    nc.scalar.dma_start(out_d[tm * P:(tm + 1) * P, :], o[:])

    nc.compile()
    return nc


def _pack_inputs(sorted_x, w12, w3, starts, per, dtype="f32r"):
    """Host-side shard + transpose packing for each core."""
    if dtype == "f32r":
        np_dt = np.float32
    elif dtype == "fp16":
        np_dt = np.float16
    else:
        import ml_dtypes
        np_dt = ml_dtypes.bfloat16
    in_maps = []
    for e in range(N_EXPERTS):
        xs = sorted_x[starts[e]:starts[e] + per]                 # (1024, 512)
        xT = np.ascontiguousarray(xs.T).reshape(4, 128, TOKENS_PER_EXPERT)
        # w12c[jc][p, ic*128+j] = w12[e][jc*128+j, ic*128+p]
        w12c = np.ascontiguousarray(
            w12[e].reshape(16, 128, 4, 128).transpose(0, 3, 2, 1)
        ).reshape(16, 128, 512)
        # pair-major: w12p[p] = stack(w12c[p], w12c[p+8]) -> (8, 2, 128, 512)
        w12p = np.ascontiguousarray(np.stack([w12c[:8], w12c[8:]], axis=1))
        # w3c[jc][k, n] = w3[e][n, jc*128+k]; grouped 4 chunks per DMA
        w3g = np.ascontiguousarray(
            w3[e].reshape(512, 8, 128).transpose(1, 2, 0)
        ).reshape(2, 4, 128, 512)
        in_maps.append({
            "xT": xT.astype(np_dt, copy=False),
            "w12p": w12p.astype(np_dt, copy=False),
            "w3g": w3g.astype(np_dt, copy=False),
        })
    return in_maps


def _reference_numpy(sorted_x, w12, w3, expert_starts, expert_ends):
    """Exact fallback for non-canonical segment layouts."""
    x = sorted_x.astype(np.float32)
    T = x.shape[0]
    out = np.zeros((T, w3.shape[1]), dtype=np.float32)
    tok = np.arange(T)
    for e in range(w12.shape[0]):
        m = (tok >= expert_starts[e]) & (tok < expert_ends[e])
        if not m.any():
            continue
        h12 = x[m] @ w12[e].T
        h1, h2 = h12[:, :HIDDEN], h12[:, HIDDEN:]
        hact = (h1 / (1.0 + np.exp(-h1))) * h2
        out[m] += hact @ w3[e].T
    return out


def kernel(sorted_x, w12, w3, expert_starts, expert_ends):
    sorted_x = np.asarray(sorted_x)
    w12 = np.asarray(w12)
    w3 = np.asarray(w3)
    starts = np.asarray(expert_starts).astype(np.int64)
    ends = np.asarray(expert_ends).astype(np.int64)

    T = sorted_x.shape[0]
    E = w12.shape[0]
    per = T // E
    canonical = (
        E == N_EXPERTS
        and T == N_EXPERTS * TOKENS_PER_EXPERT
        and sorted_x.shape[1] == D_MODEL
        and w12.shape[1:] == (2 * HIDDEN, D_MODEL)
        and w3.shape[1:] == (D_MODEL, HIDDEN)
        and bool(np.all(starts == np.arange(E, dtype=np.int64) * per))
        and bool(np.all(ends == starts + per))
    )
    if not canonical:
        return _reference_numpy(sorted_x, w12, w3, starts, ends)

    from concourse.bass_utils import run_bass_kernel_spmd

    if "nc" not in _CACHE:
        _CACHE["nc"] = _build_program()
    nc = _CACHE["nc"]

    in_maps = _pack_inputs(sorted_x, w12, w3, starts, per)
    res = run_bass_kernel_spmd(nc, in_maps, list(range(N_CORES)))
    out = np.empty((T, D_MODEL), dtype=np.float32)
    for e in range(N_EXPERTS):
        out[e * per:(e + 1) * per] = res.results[e]["out"]
    return out



# revision 2
# speedup vs baseline: 1.1235x; 1.1235x over previous
"""Grouped-expert SwiGLU FFN (MoE) kernel for 8 Trainium2 NeuronCores.

Problem: 8 experts, tokens pre-sorted into contiguous equal segments.
  sorted_x: (8192, 512) f32, w12: (8, 2048, 512) f32, w3: (8, 512, 1024) f32
  out[t] = SwiGLU(x[t] @ w12[e].T) @ w3[e].T   for t in expert e's segment

Sharding: expert parallelism. Core e gets its 1024-token segment plus
w12[e]/w3[e]; no collectives. Host pre-transposes operands so the
contraction dim lands on SBUF partitions (no on-chip transposes):

  GEMM1 (contract d_model=512, 4 chunks of 128):
    lhsT = w12[e].T chunk (i=128, j=128)  [stationary]
    rhs  = x_seg.T  chunk (i=128, t=512)  [moving]
    psum (j=128, t=512) accumulated over 4 i-chunks  -> h12^T orientation
  SwiGLU: silu(h1^T) * h2^T elementwise in (j, t) layout (ACT + DVE)
  GEMM2 (contract hidden=1024, 8 chunks of 128):
    lhsT = h^T chunk   (j=128, t=128)  [stationary]
    rhs  = w3[e].T chunk (j=128, o=512) [moving]
    psum (t=128, o=512) accumulated over 8 j-chunks -> natural (t, o) output

All operands are bf16 (same PE rate as f32r, half the DMA bytes; rel
err ~1e-3 vs the 2e-2 gate). The critical path is the Tensor engine:
it is 100%-busy once started, so the kernel (a) splits input loads
across both HWDGE queues (SP carries x+w3, ACT carries w12) with
issue order = consumption order so the first GEMM1 chain's operands
land ~5us earlier than a single-queue stream, and (b) runs a block of
dependency-free warm-up matmuls on zeroed scratch during the load
wait so the PE p-state ramp (0.65/1.2GHz -> 2.4GHz after ~3us busy)
is paid before real data arrives. Output is stored bf16 (halves the
tail store) and upcast on the host.
"""

import numpy as np

N_EXPERTS = 8
D_MODEL = 512
HIDDEN = 1024
TOKENS_PER_EXPERT = 1024
N_CORES = 8

_CACHE = {}


def _build_program(repeat=1, nwarm=10):
    import concourse.mybir as mybir
    import concourse.tile as tile
    from concourse import bacc

    f32 = mybir.dt.float32
    din = mybir.dt.bfloat16
    P = 128
    IC = D_MODEL // P            # 4 chunks of d_model
    JC = HIDDEN // P             # 8 chunk-pairs of 2*hidden / chunks of hidden
    WG = 2                       # w3 DMA groups
    TB = 512                     # token block (moving free dim)
    NT = TOKENS_PER_EXPERT // TB  # 2 token blocks
    TM = TOKENS_PER_EXPERT // P  # 8 output token chunks

    nc = bacc.Bacc(None, target_bir_lowering=False)

    # host-packed layouts (see _pack_inputs)
    xh_d = nc.dram_tensor("xh", [NT, IC, P, TB], din, kind="ExternalInput")
    w12_d = nc.dram_tensor("w12p", [JC, 2, P, IC * P], din, kind="ExternalInput")
    w3_d = nc.dram_tensor("w3g", [WG, JC // WG, P, D_MODEL], din, kind="ExternalInput")
    out_d = nc.dram_tensor("out", [TOKENS_PER_EXPERT, D_MODEL], din, kind="ExternalOutput")

    with tile.TileContext(nc) as tc:
        with (
            tc.tile_pool(name="persist", bufs=2) as persist,
            tc.tile_pool(name="work", bufs=3) as work,
            tc.tile_pool(name="ps1", bufs=3, space="PSUM") as ps1,
            tc.tile_pool(name="ps2", bufs=2, space="PSUM") as ps2,
        ):
            # PE warm-up: zeroed scratch, no DMA deps; ramps the PE clock
            # while the first input DMAs are in flight. Borrows the ps_o
            # buffers (free until GEMM2).
            warm = work.tile([P, TB], din, tag="warm", bufs=1, name="warm")
            nc.vector.memset(warm[:], 0)
            for k in range(nwarm):
                pw = ps2.tile([P, D_MODEL], f32, tag="ps_o", name=f"warm{k}")
                nc.tensor.matmul(pw[:], warm[:, 0:P], warm[:],
                                 start=True, stop=True)

            for it in range(repeat):
                # ---- loads: two HWDGE queues, issue order = consumption ----
                # SP queue: x halves (ic-pair split for the first bite), w3.
                # ACT queue: w12 chunk-pairs in p order.
                xh, w12, w3 = [], [], []
                for tb in range(NT):
                    t = persist.tile([P, IC, TB], din, tag=f"xh{tb}",
                                     name=f"i{it}_xh{tb}")
                    nc.sync.dma_start(
                        t[:, 0:2, :], xh_d[tb, 0:2].rearrange("i p t -> p i t"))
                    nc.sync.dma_start(
                        t[:, 2:4, :], xh_d[tb, 2:4].rearrange("i p t -> p i t"))
                    xh.append(t)
                    if tb == 0:
                        for p in range(JC):
                            w = persist.tile([P, 2, IC * P], din, tag=f"w12_{p}",
                                             name=f"i{it}_w12_{p}")
                            nc.scalar.dma_start(
                                w[:], w12_d[p].rearrange("h p m -> p h m"))
                            w12.append(w)
                for g in range(WG):
                    t = persist.tile([P, JC // WG, D_MODEL], din, tag=f"w3_{g}",
                                     name=f"i{it}_w3_{g}")
                    nc.sync.dma_start(t[:], w3_d[g].rearrange("q p m -> p q m"))
                    w3.append(t)
                h = [persist.tile([P, TOKENS_PER_EXPERT], din, tag=f"h{p}", bufs=1,
                                  name=f"i{it}_h{p}") for p in range(JC)]

                # ---- GEMM1 + SwiGLU ----
                for p in range(JC):
                    for tb in range(NT):
                        tsl = slice(tb * TB, (tb + 1) * TB)
                        ps_a = ps1.tile([P, TB], f32, tag="ps_a", name=f"i{it}_a{p}_{tb}")
                        ps_b = ps1.tile([P, TB], f32, tag="ps_b", name=f"i{it}_b{p}_{tb}")
                        for ic in range(IC):
                            nc.tensor.matmul(
                                ps_a[:], w12[p][:, 0, ic * P:(ic + 1) * P],
                                xh[tb][:, ic, :], start=(ic == 0), stop=(ic == IC - 1))
                        for ic in range(IC):
                            nc.tensor.matmul(
                                ps_b[:], w12[p][:, 1, ic * P:(ic + 1) * P],
                                xh[tb][:, ic, :], start=(ic == 0), stop=(ic == IC - 1))
                        s = work.tile([P, TB], f32, tag="silu", name=f"i{it}_s{p}_{tb}")
                        nc.scalar.activation(s[:], ps_a[:],
                                             mybir.ActivationFunctionType.Silu)
                        nc.vector.tensor_tensor(h[p][:, tsl], s[:], ps_b[:],
                                                mybir.AluOpType.mult)

                # ---- GEMM2 ----
                for tm in range(TM):
                    ps_o = ps2.tile([P, D_MODEL], f32, tag="ps_o", name=f"i{it}_o{tm}")
                    for jc in range(JC):
                        nc.tensor.matmul(
                            ps_o[:], h[jc][:, tm * P:(tm + 1) * P],
                            w3[jc // (JC // WG)][:, jc % (JC // WG), :],
                            start=(jc == 0), stop=(jc == JC - 1))
                    o = work.tile([P, D_MODEL], din, tag="o", name=f"i{it}_oo{tm}")
                    nc.vector.tensor_copy(o[:], ps_o[:])
                    nc.scalar.dma_start(out_d[tm * P:(tm + 1) * P, :], o[:])

    nc.compile()
    return nc


def _pack_inputs(sorted_x, w12, w3, starts, per):
    """Host-side shard + transpose packing for each core (bf16)."""
    import ml_dtypes
    np_dt = ml_dtypes.bfloat16
    in_maps = []
    for e in range(N_EXPERTS):
        xs = sorted_x[starts[e]:starts[e] + per]                 # (1024, 512)
        # xh[tb, ic, p, u] = x_seg[tb*512+u, ic*128+p]
        xh = np.ascontiguousarray(
            xs.T.reshape(4, 128, 2, 512).transpose(2, 0, 1, 3))
        # w12c[jc][p, ic*128+j] = w12[e][jc*128+j, ic*128+p]
        w12c = np.ascontiguousarray(
            w12[e].reshape(16, 128, 4, 128).transpose(0, 3, 2, 1)
        ).reshape(16, 128, 512)
        # pair-major: w12p[p] = stack(w12c[p], w12c[p+8]) -> (8, 2, 128, 512)
        w12p = np.ascontiguousarray(np.stack([w12c[:8], w12c[8:]], axis=1))
        # w3c[jc][k, n] = w3[e][n, jc*128+k]; grouped 4 chunks per DMA
        w3g = np.ascontiguousarray(
            w3[e].reshape(512, 8, 128).transpose(1, 2, 0)
        ).reshape(2, 4, 128, 512)
        in_maps.append({
            "xh": xh.astype(np_dt),
            "w12p": w12p.astype(np_dt),
            "w3g": w3g.astype(np_dt),
        })
    return in_maps


def _reference_numpy(sorted_x, w12, w3, expert_starts, expert_ends):
    """Exact fallback for non-canonical segment layouts."""
    x = sorted_x.astype(np.float32)
    T = x.shape[0]
    out = np.zeros((T, w3.shape[1]), dtype=np.float32)
    tok = np.arange(T)
    for e in range(w12.shape[0]):
        m = (tok >= expert_starts[e]) & (tok < expert_ends[e])
        if not m.any():
            continue
        h12 = x[m] @ w12[e].T
        h1, h2 = h12[:, :HIDDEN], h12[:, HIDDEN:]
        hact = (h1 / (1.0 + np.exp(-h1))) * h2
        out[m] += hact @ w3[e].T
    return out


def kernel(sorted_x, w12, w3, expert_starts, expert_ends):
    sorted_x = np.asarray(sorted_x)
    w12 = np.asarray(w12)
    w3 = np.asarray(w3)
    starts = np.asarray(expert_starts).astype(np.int64)
    ends = np.asarray(expert_ends).astype(np.int64)

    T = sorted_x.shape[0]
    E = w12.shape[0]
    per = T // E
    canonical = (
        E == N_EXPERTS
        and T == N_EXPERTS * TOKENS_PER_EXPERT
        and sorted_x.shape[1] == D_MODEL
        and w12.shape[1:] == (2 * HIDDEN, D_MODEL)
        and w3.shape[1:] == (D_MODEL, HIDDEN)
        and bool(np.all(starts == np.arange(E, dtype=np.int64) * per))
        and bool(np.all(ends == starts + per))
    )
    if not canonical:
        return _reference_numpy(sorted_x, w12, w3, starts, ends)

    from concourse.bass_utils import run_bass_kernel_spmd

    if "nc" not in _CACHE:
        _CACHE["nc"] = _build_program()
    nc = _CACHE["nc"]

    in_maps = _pack_inputs(sorted_x, w12, w3, starts, per)
    res = run_bass_kernel_spmd(nc, in_maps, list(range(N_CORES)))
    out = np.empty((T, D_MODEL), dtype=np.float32)
    for e in range(N_EXPERTS):
        out[e * per:(e + 1) * per] = res.results[e]["out"].astype(np.float32)
    return out


# revision 5
# speedup vs baseline: 1.1272x; 1.0033x over previous
"""Grouped-expert SwiGLU FFN (MoE) kernel for 8 Trainium2 NeuronCores.

Problem: 8 experts, tokens pre-sorted into contiguous equal segments.
  sorted_x: (8192, 512) f32, w12: (8, 2048, 512) f32, w3: (8, 512, 1024) f32
  out[t] = SwiGLU(x[t] @ w12[e].T) @ w3[e].T   for t in expert e's segment

Sharding: expert parallelism. Core e gets its 1024-token segment plus
w12[e]/w3[e]; no collectives. Host pre-transposes operands so the
contraction dim lands on SBUF partitions (no on-chip transposes):

  GEMM1 (contract d_model=512, 4 chunks of 128):
    lhsT = w12[e].T chunk (i=128, j=128)  [stationary]
    rhs  = x_seg.T  chunk (i=128, t=512)  [moving]
    psum (j=128, t=512) accumulated over 4 i-chunks  -> h12^T orientation
  SwiGLU: silu(h1^T) * h2^T elementwise in (j, t) layout (ACT + DVE)
  GEMM2 (contract hidden=1024, 8 chunks of 128):
    lhsT = h^T chunk   (j=128, t=128)  [stationary]
    rhs  = w3[e].T chunk (j=128, o=512) [moving]
    psum (t=128, o=512) accumulated over 8 j-chunks -> natural (t, o) output

All operands are bf16 (same PE rate as f32r, half the DMA bytes; rel
err ~1e-3 vs the 2e-2 gate). The critical path is the Tensor engine:
it is 100%-busy once started, so the kernel (a) splits input loads
across both HWDGE queues (SP carries x+w3, ACT carries w12) with
issue order = consumption order so the first GEMM1 chain's operands
land ~5us earlier than a single-queue stream, and (b) runs a block of
dependency-free warm-up matmuls on zeroed scratch during the load
wait so the PE p-state ramp (0.65/1.2GHz -> 2.4GHz after ~3us busy)
is paid before real data arrives. Output is stored bf16 (halves the
tail store) and upcast on the host.
"""

import numpy as np

N_EXPERTS = 8
D_MODEL = 512
HIDDEN = 1024
TOKENS_PER_EXPERT = 1024
N_CORES = 8

_CACHE = {}


def _build_program(repeat=1, nwarm=8):
    import concourse.mybir as mybir
    import concourse.tile as tile
    from concourse import bacc

    f32 = mybir.dt.float32
    din = mybir.dt.bfloat16
    P = 128
    IC = D_MODEL // P            # 4 chunks of d_model
    JC = HIDDEN // P             # 8 chunk-pairs of 2*hidden / chunks of hidden
    WG = 2                       # w3 DMA groups
    TB = 512                     # token block (moving free dim)
    NT = TOKENS_PER_EXPERT // TB  # 2 token blocks
    TM = TOKENS_PER_EXPERT // P  # 8 output token chunks

    nc = bacc.Bacc(None, target_bir_lowering=False)

    # host-packed layouts (see _pack_inputs)
    xh_d = nc.dram_tensor("xh", [NT, IC, P, TB], din, kind="ExternalInput")
    w12_d = nc.dram_tensor("w12p", [JC, 2, P, IC * P], din, kind="ExternalInput")
    w3_d = nc.dram_tensor("w3g", [WG, JC // WG, P, D_MODEL], din, kind="ExternalInput")
    out_d = nc.dram_tensor("out", [TOKENS_PER_EXPERT, D_MODEL], din, kind="ExternalOutput")

    with tile.TileContext(nc) as tc:
        with (
            tc.tile_pool(name="persist", bufs=2) as persist,
            tc.tile_pool(name="work", bufs=3) as work,
            tc.tile_pool(name="ps1", bufs=3, space="PSUM") as ps1,
            tc.tile_pool(name="ps2", bufs=2, space="PSUM") as ps2,
        ):
            # PE warm-up: zeroed scratch, no DMA deps; ramps the PE clock
            # while the first input DMAs are in flight. Borrows the ps_o
            # buffers (free until GEMM2).
            warm = work.tile([P, TB], din, tag="warm", bufs=1, name="warm")
            nc.vector.memset(warm[:], 0)
            for k in range(nwarm):
                pw = ps2.tile([P, D_MODEL], f32, tag="ps_o", name=f"warm{k}")
                nc.tensor.matmul(pw[:], warm[:, 0:P], warm[:],
                                 start=True, stop=True)

            for it in range(repeat):
                # ---- loads ----
                # One global stream in consumption order, alternated across
                # the two HWDGE queues (SP / ACT) so both sequencers issue in
                # parallel and descriptors from the head of the stream share
                # all 16 DMA engines. First bites are small (ic-pair halves
                # of x, a/b halves of w12[0]) so GEMM1 starts ~2us sooner.
                xh = [persist.tile([P, IC, TB], din, tag=f"xh{tb}",
                                   name=f"i{it}_xh{tb}") for tb in range(NT)]
                w12 = [persist.tile([P, 2, IC * P], din, tag=f"w12_{p}",
                                    name=f"i{it}_w12_{p}") for p in range(JC)]
                w3 = [persist.tile([P, JC // WG, D_MODEL], din, tag=f"w3_{g}",
                                   name=f"i{it}_w3_{g}") for g in range(WG)]
                stream = [
                    (xh[0][:, 0:2, :], xh_d[0, 0:2].rearrange("i p t -> p i t")),
                    (w12[0][:, 0, :], w12_d[0, 0]),
                    (xh[0][:, 2:4, :], xh_d[0, 2:4].rearrange("i p t -> p i t")),
                    (w12[0][:, 1, :], w12_d[0, 1]),
                    (xh[1][:, 0:2, :], xh_d[1, 0:2].rearrange("i p t -> p i t")),
                    (xh[1][:, 2:4, :], xh_d[1, 2:4].rearrange("i p t -> p i t")),
                ] + [
                    (w12[p][:], w12_d[p].rearrange("h p m -> p h m"))
                    for p in range(1, JC)
                ] + [
                    (w3[g][:], w3_d[g].rearrange("q p m -> p q m"))
                    for g in range(WG)
                ]
                for pos, (dst, src) in enumerate(stream):
                    eng = nc.sync if pos % 2 == 0 else nc.scalar
                    eng.dma_start(dst, src)
                h = [persist.tile([P, TOKENS_PER_EXPERT], din, tag=f"h{p}", bufs=1,
                                  name=f"i{it}_h{p}") for p in range(JC)]

                # ---- GEMM1 + SwiGLU ----
                for p in range(JC):
                    for tb in range(NT):
                        tsl = slice(tb * TB, (tb + 1) * TB)
                        ps_a = ps1.tile([P, TB], f32, tag="ps_a", name=f"i{it}_a{p}_{tb}")
                        ps_b = ps1.tile([P, TB], f32, tag="ps_b", name=f"i{it}_b{p}_{tb}")
                        for ic in range(IC):
                            nc.tensor.matmul(
                                ps_a[:], w12[p][:, 0, ic * P:(ic + 1) * P],
                                xh[tb][:, ic, :], start=(ic == 0), stop=(ic == IC - 1))
                        for ic in range(IC):
                            nc.tensor.matmul(
                                ps_b[:], w12[p][:, 1, ic * P:(ic + 1) * P],
                                xh[tb][:, ic, :], start=(ic == 0), stop=(ic == IC - 1))
                        s = work.tile([P, TB], f32, tag="silu", name=f"i{it}_s{p}_{tb}")
                        nc.scalar.activation(s[:], ps_a[:],
                                             mybir.ActivationFunctionType.Silu)
                        nc.vector.tensor_tensor(h[p][:, tsl], s[:], ps_b[:],
                                                mybir.AluOpType.mult)

                # ---- GEMM2 ----
                # Last token chunk is split into two column halves so the
                # copy+store of the first half overlaps the second half's
                # matmul chain, shortening the kernel tail.
                for tm in range(TM):
                    ps_o = ps2.tile([P, D_MODEL], f32, tag="ps_o", name=f"i{it}_o{tm}")
                    rsl = slice(tm * P, (tm + 1) * P)
                    if tm < TM - 1:
                        for jc in range(JC):
                            nc.tensor.matmul(
                                ps_o[:], h[jc][:, rsl],
                                w3[jc // (JC // WG)][:, jc % (JC // WG), :],
                                start=(jc == 0), stop=(jc == JC - 1))
                        o = work.tile([P, D_MODEL], din, tag="o", name=f"i{it}_oo{tm}")
                        nc.vector.tensor_copy(o[:], ps_o[:])
                        nc.scalar.dma_start(out_d[rsl, :], o[:])
                    else:
                        HB = D_MODEL // 2
                        for hb in range(2):
                            csl = slice(hb * HB, (hb + 1) * HB)
                            for jc in range(JC):
                                nc.tensor.matmul(
                                    ps_o[:, csl], h[jc][:, rsl],
                                    w3[jc // (JC // WG)][:, jc % (JC // WG), csl],
                                    start=(jc == 0), stop=(jc == JC - 1))
                            o = work.tile([P, HB], din, tag=f"o7_{hb}",
                                          name=f"i{it}_oo{tm}_{hb}")
                            nc.vector.tensor_copy(o[:], ps_o[:, csl])
                            nc.scalar.dma_start(out_d[rsl, csl], o[:])

    nc.compile()
    return nc


def _pack_inputs(sorted_x, w12, w3, starts, per):
    """Host-side shard + transpose packing for each core (bf16)."""
    import ml_dtypes
    np_dt = ml_dtypes.bfloat16
    in_maps = []
    for e in range(N_EXPERTS):
        xs = sorted_x[starts[e]:starts[e] + per]                 # (1024, 512)
        # xh[tb, ic, p, u] = x_seg[tb*512+u, ic*128+p]
        xh = np.ascontiguousarray(
            xs.T.reshape(4, 128, 2, 512).transpose(2, 0, 1, 3))
        # w12c[jc][p, ic*128+j] = w12[e][jc*128+j, ic*128+p]
        w12c = np.ascontiguousarray(
            w12[e].reshape(16, 128, 4, 128).transpose(0, 3, 2, 1)
        ).reshape(16, 128, 512)
        # pair-major: w12p[p] = stack(w12c[p], w12c[p+8]) -> (8, 2, 128, 512)
        w12p = np.ascontiguousarray(np.stack([w12c[:8], w12c[8:]], axis=1))
        # w3c[jc][k, n] = w3[e][n, jc*128+k]; grouped 4 chunks per DMA
        w3g = np.ascontiguousarray(
            w3[e].reshape(512, 8, 128).transpose(1, 2, 0)
        ).reshape(2, 4, 128, 512)
        in_maps.append({
            "xh": xh.astype(np_dt),
            "w12p": w12p.astype(np_dt),
            "w3g": w3g.astype(np_dt),
        })
    return in_maps


def _reference_numpy(sorted_x, w12, w3, expert_starts, expert_ends):
    """Exact fallback for non-canonical segment layouts."""
    x = sorted_x.astype(np.float32)
    T = x.shape[0]
    out = np.zeros((T, w3.shape[1]), dtype=np.float32)
    tok = np.arange(T)
    for e in range(w12.shape[0]):
        m = (tok >= expert_starts[e]) & (tok < expert_ends[e])
        if not m.any():
            continue
        h12 = x[m] @ w12[e].T
        h1, h2 = h12[:, :HIDDEN], h12[:, HIDDEN:]
        hact = (h1 / (1.0 + np.exp(-h1))) * h2
        out[m] += hact @ w3[e].T
    return out


def kernel(sorted_x, w12, w3, expert_starts, expert_ends):
    sorted_x = np.asarray(sorted_x)
    w12 = np.asarray(w12)
    w3 = np.asarray(w3)
    starts = np.asarray(expert_starts).astype(np.int64)
    ends = np.asarray(expert_ends).astype(np.int64)

    T = sorted_x.shape[0]
    E = w12.shape[0]
    per = T // E
    canonical = (
        E == N_EXPERTS
        and T == N_EXPERTS * TOKENS_PER_EXPERT
        and sorted_x.shape[1] == D_MODEL
        and w12.shape[1:] == (2 * HIDDEN, D_MODEL)
        and w3.shape[1:] == (D_MODEL, HIDDEN)
        and bool(np.all(starts == np.arange(E, dtype=np.int64) * per))
        and bool(np.all(ends == starts + per))
    )
    if not canonical:
        return _reference_numpy(sorted_x, w12, w3, starts, ends)

    from concourse.bass_utils import run_bass_kernel_spmd

    if "nc" not in _CACHE:
        _CACHE["nc"] = _build_program()
    nc = _CACHE["nc"]

    in_maps = _pack_inputs(sorted_x, w12, w3, starts, per)
    res = run_bass_kernel_spmd(nc, in_maps, list(range(N_CORES)))
    out = np.empty((T, D_MODEL), dtype=np.float32)
    for e in range(N_EXPERTS):
        out[e * per:(e + 1) * per] = res.results[e]["out"].astype(np.float32)
    return out
